# revision 8
# baseline (speedup 1.0000x reference)
"""MultiHeadAttention with Hawkes temporal bias (TA-NFT) on 8 Trainium2 cores.

Sharding: head-parallel — core i computes head i for all 32 batches, producing
the partial final projection out_h @ Wf_h; the host sums the 8 partials
(plus bias, added on core 0) to unshard.

Per (core, batch) the computation uses feature-on-partition ("transposed")
layouts throughout so that every matmul contraction dim lands on partitions
with no on-device transposes:
  qT[e,q]      = sum_d Wq[d,e]^T       @ queryT[d,q]
  scoresT[k,q] = sum_e contextT[e,k]^T @ qT[e,q]
  pT           = exp(scoresT + exp(-beta*dt + ln eps))      (no max-sub; |s|<~60)
  rowsum[*,q]  = ones[k,128]^T @ pT[k,q]   (PE trick: sum replicated on all 128
                                            partitions, so the reciprocal can be
                                            applied as a plain tensor_tensor mul)
  wT           = pT * recip(rowsum)
  mixT[d,q]    = sum_k context[k,d]^T  @ wT[k,q]
  z[f,q]       = sum_d WoutM[d,f]^T @ mixT + sum_e WoutQ[e,f]^T @ qT
  outhT        = tanh(z)
  finT[o,q]    = sum_f Wf[f,o]^T @ outhT[f,q]  (+ bf share)
All matmul operands are bitcast to float32r (fp22 multiply, fp32 accumulate)
which runs at full PE rate for 512-wide moving operands.
"""

import os

import numpy as np

H, D, B, Q, K = 8, 512, 32, 512, 512
P = 128  # SBUF partitions
C = 4  # 512 / 128 chunks
N_CORES = 8
N_B = int(os.environ.get("KERNEL_NB", str(B)))  # batches per core (32 = full)


def _split_sync_waits(nc, mybir, max_waits=1):
    """This container's walrus build rejects any instruction carrying more
    than one semaphore wait ("Too many sync wait commands"). Move overflow
    waits onto preceding same-engine NoOp carriers (same engine program
    order => semantically identical)."""
    uid = 0
    for fn in nc.m.functions:
        for bb in fn.blocks:
            out = []
            for inst in bb.instructions:
                si = inst.sync_info
                if si is not None and si.on_wait and len(si.on_wait) > max_waits:
                    waits = list(si.on_wait)
                    for i in range(0, len(waits) - max_waits, max_waits):
                        carrier = mybir.InstNoOp(name=f"I-waitsplit-{uid}")
                        uid += 1
                        carrier.engine = inst.engine
                        carrier.sync_info = mybir.SyncInfo(
                            on_wait=waits[i : i + max_waits], on_update=[]
                        )
                        out.append(carrier)
                    si.on_wait = waits[len(waits) - max_waits :]
                out.append(inst)
            bb.instructions = out


def _build_bass(n_b):
    import concourse.bass as bass
    import concourse.mybir as mybir
    import concourse.tile as tile

    f32 = mybir.dt.float32
    f32r = mybir.dt.float32r
    Act = mybir.ActivationFunctionType

    def r(ap):  # fp32r view (matmul operands / rounding-producer outputs)
        return ap.bitcast(f32r)

    nc = bass.Bass("TRN2", target_bir_lowering=False, debug=False)

    dt_in = lambda name, shape: nc.dram_tensor(
        name, shape, f32, kind="ExternalInput"
    ).ap()
    g_qT = dt_in("qT", [B, P, C * Q])  # queryT packed: [b, p, c*Q+q] = query[b,q,c*128+p]
    g_ctxT = dt_in("ctxT", [B, P, C * K])  # contextT packed (e on partitions)
    g_ctx = dt_in("ctx", [B, P, C * D])  # context packed (k on partitions)
    g_dtT = dt_in("dtT", [B, P, C * Q])  # delta_t^T packed (k on partitions)
    g_wq = dt_in("wq", [P, C * D])
    g_woM = dt_in("woM", [P, C * D])
    g_woQ = dt_in("woQ", [P, C * D])
    g_wf = dt_in("wf", [P, C * D])
    g_scal = dt_in("scal", [P, 2 * B])  # [:, b] = -beta ; [:, B+b] = ln eps
    g_bias = dt_in("bias", [P, C])  # bf share (core 0) else zeros
    g_out = nc.dram_tensor("out", [B, P, C * Q], f32, kind="ExternalOutput").ap()

    with tile.TileContext(nc) as tc:
        with (
            tc.tile_pool(name="wpool", bufs=1) as wpool,
            tc.tile_pool(name="stage", bufs=4) as stpool,
            tc.tile_pool(name="inpool", bufs=2) as inpool,
            tc.tile_pool(name="qopool", bufs=3) as qopool,
            tc.tile_pool(name="pmpool", bufs=3) as pmpool,
            tc.tile_pool(name="fpool", bufs=2) as fpool,
            tc.tile_pool(name="pspool", bufs=8, space="PSUM") as pspool,
        ):
            # --- resident weights: DMA chunk -> stage, round -> f32r tile ---
            # (walrus requires every writer of an fp32r matmul operand to be
            # an engine op with fp32r output; DMA/memset don't qualify.)
            wtiles = {}
            for name, g in (("wq", g_wq), ("woM", g_woM), ("woQ", g_woQ), ("wf", g_wf)):
                wt = wpool.tile([P, C, D], f32, tag=name)
                for c in range(C):
                    st = stpool.tile([P, D], f32, tag="stage")
                    nc.sync.dma_start(st, g[:, c * D : (c + 1) * D])
                    nc.vector.tensor_copy(r(wt[:, c, :]), st)
                wtiles[name] = wt
            wq_s, woM_s, woQ_s, wf_s = (
                wtiles["wq"], wtiles["woM"], wtiles["woQ"], wtiles["wf"]
            )
            ones_st = stpool.tile([P, P], f32, tag="ones_st")
            nc.vector.memset(ones_st, 1.0)
            ones_s = wpool.tile([P, P], f32, tag="ones")
            nc.vector.tensor_copy(r(ones_s), ones_st)
            scal_s = wpool.tile([P, 2 * B], f32, tag="scal")
            nc.sync.dma_start(scal_s, g_scal)
            bias_s = wpool.tile([P, C], f32, tag="bias")
            nc.sync.dma_start(bias_s, g_bias)

            for b in range(n_b):
                # --- inputs: stage + round (qT/ctxT/ctx); dtT used raw ---
                rin = {}
                for name, g, eng in (
                    ("qT", g_qT, "act"),
                    ("ctxT", g_ctxT, "dve"),
                    ("ctx", g_ctx, "dve"),
                ):
                    t = inpool.tile([P, C, Q], f32, tag=name + "_r")
                    for c in range(C):
                        st = stpool.tile([P, Q], f32, tag="stage")
                        nc.sync.dma_start(st, g[b, :, c * Q : (c + 1) * Q])
                        if eng == "act":
                            nc.scalar.copy(r(t[:, c, :]), st)
                        else:
                            nc.vector.tensor_copy(r(t[:, c, :]), st)
                    rin[name] = t
                qT_in, ctxT_in, ctx_in = rin["qT"], rin["ctxT"], rin["ctx"]
                dtT_in = inpool.tile([P, C, Q], f32, tag="dtT_in")
                nc.sync.dma_start(dtT_in, g_dtT[b].rearrange("p (c q) -> p c q", c=C))

                # ---- q projection: qT[e,q] ----
                qTs = qopool.tile([P, C, Q], f32, tag="qo")
                for e in range(C):
                    ps = pspool.tile([P, Q], f32, tag="ps")
                    for c in range(C):
                        nc.tensor.matmul(
                            ps,
                            r(wq_s[:, c, e * P : (e + 1) * P]),
                            r(qT_in[:, c, :]),
                            start=(c == 0),
                            stop=(c == C - 1),
                        )
                    nc.scalar.copy(r(qTs[:, e, :]), ps)

                # ---- scoresT[k,q] + hawkes bias + exp ----
                pk = pmpool.tile([P, C, Q], f32, tag="pm")
                for k in range(C):
                    ps = pspool.tile([P, Q], f32, tag="ps")
                    for c in range(C):
                        nc.tensor.matmul(
                            ps,
                            r(ctxT_in[:, c, k * P : (k + 1) * P]),
                            r(qTs[:, c, :]),
                            start=(c == 0),
                            stop=(c == C - 1),
                        )
                    # hawkes = exp(-beta * dt + ln eps)
                    nc.scalar.activation(
                        r(pk[:, k, :]),
                        dtT_in[:, k, :],
                        Act.Exp,
                        bias=scal_s[:, B + b : B + b + 1],
                        scale=scal_s[:, b : b + 1],
                    )
                    nc.vector.tensor_add(r(pk[:, k, :]), pk[:, k, :], ps)
                    nc.scalar.activation(r(pk[:, k, :]), pk[:, k, :], Act.Exp)

                # ---- softmax denominator (replicated across partitions) ----
                rs = pspool.tile([P, Q], f32, tag="ps")
                for k in range(C):
                    nc.tensor.matmul(
                        rs,
                        r(ones_s),
                        r(pk[:, k, :]),
                        start=(k == 0),
                        stop=(k == C - 1),
                    )
                recip = stpool.tile([P, Q], f32, tag="stage")
                nc.vector.reciprocal(recip, rs)
                for k in range(C):
                    nc.vector.tensor_mul(r(pk[:, k, :]), pk[:, k, :], recip)

                # ---- mixT[d,q] ----
                mixTs = pmpool.tile([P, C, Q], f32, tag="pm")
                for d in range(C):
                    ps = pspool.tile([P, Q], f32, tag="ps")
                    for c in range(C):
                        nc.tensor.matmul(
                            ps,
                            r(ctx_in[:, c, d * P : (d + 1) * P]),
                            r(pk[:, c, :]),
                            start=(c == 0),
                            stop=(c == C - 1),
                        )
                    nc.vector.tensor_copy(r(mixTs[:, d, :]), ps)

                # ---- z[f,q] = WoutM^T mixT + WoutQ^T qT ; tanh ----
                outhTs = qopool.tile([P, C, Q], f32, tag="qo")
                for f in range(C):
                    ps = pspool.tile([P, Q], f32, tag="ps")
                    for c in range(C):
                        nc.tensor.matmul(
                            ps,
                            r(woM_s[:, c, f * P : (f + 1) * P]),
                            r(mixTs[:, c, :]),
                            start=(c == 0),
                            stop=False,
                        )
                    for c in range(C):
                        nc.tensor.matmul(
                            ps,
                            r(woQ_s[:, c, f * P : (f + 1) * P]),
                            r(qTs[:, c, :]),
                            start=False,
                            stop=(c == C - 1),
                        )
                    nc.scalar.activation(r(outhTs[:, f, :]), ps, Act.Tanh)

                # ---- final partial: finT[o,q] = Wf^T outhT (+ bias share) ----
                fin = fpool.tile([P, C, Q], f32, tag="fin")
                for o in range(C):
                    ps = pspool.tile([P, Q], f32, tag="ps")
                    for c in range(C):
                        nc.tensor.matmul(
                            ps,
                            r(wf_s[:, c, o * P : (o + 1) * P]),
                            r(outhTs[:, c, :]),
                            start=(c == 0),
                            stop=(c == C - 1),
                        )
                    nc.vector.tensor_scalar_add(fin[:, o, :], ps, bias_s[:, o : o + 1])
                nc.sync.dma_start(g_out[b], fin.rearrange("p c q -> p (c q)"))

    _split_sync_waits(nc, mybir)
    return nc


def _pack_cp(a):
    """[B, X, Y] with X = c*128+p  ->  [B, 128, 4*Y] (c-major free dim)."""
    b, x, y = a.shape
    return np.ascontiguousarray(
        a.reshape(b, C, P, y).transpose(0, 2, 1, 3).reshape(b, P, C * y),
        dtype=np.float32,
    )


def _pack_w(w):
    """[512, X] -> [128, 4*X]."""
    x = w.shape[1]
    return np.ascontiguousarray(
        w.reshape(C, P, x).transpose(1, 0, 2).reshape(P, C * x), dtype=np.float32
    )


def _make_in_maps(query, context, delta_t, Wq, Wout, eps, beta, Wf, bf):
    query = np.asarray(query, dtype=np.float32)
    context = np.asarray(context, dtype=np.float32)
    delta_t = np.asarray(delta_t, dtype=np.float32)
    Wq = np.asarray(Wq, dtype=np.float32)
    Wout = np.asarray(Wout, dtype=np.float32)
    eps = np.asarray(eps, dtype=np.float32).reshape(H, B)
    beta = np.asarray(beta, dtype=np.float32).reshape(H, B)
    Wf = np.asarray(Wf, dtype=np.float32)
    bf = np.asarray(bf, dtype=np.float32)

    qT_pack = _pack_cp(query.transpose(0, 2, 1))
    ctxT_pack = _pack_cp(context.transpose(0, 2, 1))
    ctx_pack = _pack_cp(context)
    dtT_pack = _pack_cp(delta_t.transpose(0, 2, 1))

    ln_eps = np.log(np.maximum(eps, np.float32(1e-38)))

    in_maps = []
    for h in range(N_CORES):
        scal = np.zeros((P, 2 * B), np.float32)
        scal[:, :B] = -beta[h][None, :]
        scal[:, B:] = ln_eps[h][None, :]
        bias = (
            np.ascontiguousarray(bf.reshape(C, P).T)
            if h == 0
            else np.zeros((P, C), np.float32)
        )
        in_maps.append(
            {
                "qT": qT_pack,
                "ctxT": ctxT_pack,
                "ctx": ctx_pack,
                "dtT": dtT_pack,
                "wq": _pack_w(Wq[h]),
                "woM": _pack_w(Wout[h, :D, :]),
                "woQ": _pack_w(Wout[h, D:, :]),
                "wf": _pack_w(Wf[h * D : (h + 1) * D, :]),
                "scal": scal,
                "bias": bias,
            }
        )
    return in_maps


def _unshard(outs):
    total = outs[0].astype(np.float32)
    for c in range(1, N_CORES):
        total = total + outs[c]
    # [B, 128, 4*Q]: [b, p, c*Q+q] = partial[b, q, c*128+p]
    out = (
        total.reshape(B, P, C, Q).transpose(0, 2, 1, 3).reshape(B, D, Q)
    ).transpose(0, 2, 1)
    return np.ascontiguousarray(out, dtype=np.float32)


def kernel(query, context, delta_t, Wq, Wout, eps, beta, Wf, bf):
    from concourse.bass_utils import run_bass_kernel_spmd

    in_maps = _make_in_maps(query, context, delta_t, Wq, Wout, eps, beta, Wf, bf)
    nc = _build_bass(N_B)
    res = run_bass_kernel_spmd(nc, in_maps, core_ids=list(range(N_CORES)))
    return _unshard([res.results[c]["out"] for c in range(N_CORES)])
